# revision 15
# baseline (speedup 1.0000x reference)
import sys
sys.path.insert(0, '/opt/trn_rl_repo')
import numpy as np
import ml_dtypes

BF16 = ml_dtypes.bfloat16

N = 25000
E = 400000
NCORES = 8
NPC = 3200            # padded nodes per core (25 windows x 128)
NWIN = 25

_CACHE = {}


def _prep_weights(W_sc_s, W_sc_v, W1_s, W1_v, W_r1, W_r2, W2_s, W2_v):
    c_s, c_x = np.sin(np.pi / 8.0), np.cos(np.pi / 8.0)
    # lin1 (per-edge): x feature rows [s(64)|vx|vy|vz] -> g cols [s1(64)|v1x|v1y|v1z]
    Wnode = np.zeros((160, 160), np.float32)
    Wnode[0:64, 0:64] = W1_s / 8.0
    for c in range(3):
        Wnode[64 + 32 * c:96 + 32 * c, 64 + 32 * c:96 + 32 * c] = W1_v / np.sqrt(32.0)
    # self-connection: -> [y_s-pre(96) | y_v-pre c-major(96)] with c_s folded
    Wsc = np.zeros((160, 192), np.float32)
    Wsc[0:64, 0:96] = W_sc_s / 8.0 * c_s
    for c in range(3):
        Wsc[64 + 32 * c:96 + 32 * c, 96 + 32 * c:128 + 32 * c] = \
            W_sc_v / np.sqrt(32.0) * c_s
    Wr1p = (W_r1 / np.sqrt(12.0)).astype(np.float32)
    # radial-2: [100, 640], b-major blocks of 160 = [ss(64)|vvx(32)|vvy(32)|vvz(32)]
    w1 = W_r2[:, 0:64] / 10.0
    w2 = W_r2[:, 64:128] / 10.0
    w3 = W_r2[:, 128:160] / 10.0
    w4 = W_r2[:, 160:192] / 10.0
    w5 = W_r2[:, 192:224] / 10.0
    Wr2p = np.zeros((100, 640), np.float32)
    Wr2p[:, 0:64] = w1
    for c in range(3):
        Wr2p[:, 64 + 32 * c:96 + 32 * c] = w3
    for b in range(1, 4):
        o = 160 * b
        Wr2p[:, o:o + 64] = w2
        for cp in range(3):
            Wr2p[:, o + 64 + 32 * cp:o + 96 + 32 * cp] = w4 if cp == b - 1 else w5
    # lin2 over acc(640) -> yp cols [y_s(96) | y_v c-major(96) | pad(64)]
    k = c_x / 4.0
    ks = k / np.sqrt(96.0)
    kv = k / np.sqrt(128.0)
    eps = np.zeros((3, 3, 3), np.float32)
    eps[0, 1, 2] = eps[1, 2, 0] = eps[2, 0, 1] = 1.0
    eps[0, 2, 1] = eps[1, 0, 2] = eps[2, 1, 0] = -1.0
    W2p = np.zeros((640, 192), np.float32)
    W2p[0:64, 0:96] = W2_s[0:64] * ks                       # m0a
    for c in range(3):
        W2p[64 + 32 * c:96 + 32 * c, 96 + 32 * c:128 + 32 * c] = W2_v[64:96] * kv  # m1b
    for c in range(3):                                      # attr = ve_c
        o = 160 * (c + 1)
        W2p[o:o + 64, 96 + 32 * c:128 + 32 * c] = W2_v[0:64] * kv                  # m1a
        for cp in range(3):
            r = o + 64 + 32 * cp
            if cp == c:
                W2p[r:r + 32, 0:96] = W2_s[64:96] * ks / np.sqrt(3.0)              # m0b
            else:
                i = 3 - c - cp
                sgn = eps[i, cp, c]
                W2p[r:r + 32, 96 + 32 * i:128 + 32 * i] = \
                    W2_v[96:128] * kv * sgn / np.sqrt(2.0)                          # m1c
    return (Wnode.astype(BF16), Wsc.astype(BF16), Wr1p.astype(BF16),
            Wr2p.astype(BF16), W2p.astype(BF16))


def _assign_slots(edge_dst):
    """Globally bin nodes into 8 cores x 25 windows x 128 slots, greedily
    balancing edge count per window (nodes in degree-descending order)."""
    NW = NCORES * NWIN
    deg = np.bincount(edge_dst, minlength=N)
    order = np.argsort(-deg, kind='stable')
    wsum = np.zeros(NW, np.int64)
    wcnt = np.zeros(NW, np.int64)
    core = np.empty(N, np.int64)
    slot = np.empty(N, np.int64)
    BIG = 1 << 40
    for n in order:
        w = int(np.argmin(np.where(wcnt < 128, wsum, BIG)))
        core[n] = w // NWIN
        slot[n] = (w % NWIN) * 128 + wcnt[w]
        wsum[w] += deg[n]
        wcnt[w] += 1
    return core, slot, wsum


def _prep_core(c, x, edge_src, edge_dst, edge_attr, edge_scalars, WT,
               core, slot):
    xrow = np.concatenate([np.arange(64), 64 + 3 * np.arange(32),
                           65 + 3 * np.arange(32), 66 + 3 * np.arange(32)])
    sel = np.nonzero(core[edge_dst] == c)[0]
    eslot = slot[edge_dst[sel]]
    win = eslot >> 7
    order = np.argsort(win, kind='stable')
    sel = sel[order]
    eslot = eslot[order]
    win = win[order]

    TW = WT * 128
    EP = NWIN * TW
    xg_p = np.zeros((EP, 160), np.float32)
    es_p = np.zeros((EP, 12), np.float32)
    ea_p = np.zeros((EP, 4), np.float32)
    col_p = np.full(EP, -1.0, np.float32)
    for w in range(NWIN):
        m = win == w
        ew = sel[m]
        k = ew.size
        o = w * TW
        xg_p[o:o + k] = x[edge_src[ew]][:, xrow]
        es_p[o:o + k] = edge_scalars[ew]
        ea_p[o:o + k] = edge_attr[ew]
        col_p[o:o + k] = (eslot[m] & 127).astype(np.float32)

    T = EP // 128
    xgT = np.ascontiguousarray(xg_p.T).astype(BF16)
    esT = np.ascontiguousarray(es_p.T).astype(BF16)
    eaT = np.ascontiguousarray(
        ea_p.reshape(T, 128, 4).transpose(1, 0, 2).reshape(128, T * 4)).astype(BF16)
    O = np.zeros((T, 128, 128), BF16)
    ei = np.nonzero(col_p >= 0)[0]
    O[ei // 128, ei % 128, col_p[ei].astype(np.int64)] = 1.0
    ohT = np.ascontiguousarray(O.transpose(1, 0, 2).reshape(128, T * 128))
    own = np.nonzero(core == c)[0]
    xsc = np.zeros((NPC, 160), np.float32)
    xsc[slot[own]] = x[own][:, xrow]
    xscT = np.ascontiguousarray(xsc.T).astype(BF16)
    return dict(xgT=xgT, esT=esT, eaT=eaT, ohT=ohT, xscT=xscT)


def _build_program(WT):
    import concourse.bass as bass
    import concourse.tile as tile
    from concourse import bacc, mybir

    f32 = mybir.dt.float32
    bf16 = mybir.dt.bfloat16
    i32 = mybir.dt.int32
    AF = mybir.ActivationFunctionType
    MUL = mybir.AluOpType.mult
    TW = WT * 128
    EP = NWIN * TW

    nc = bacc.Bacc("TRN2", num_devices=NCORES, debug=False)
    xgT_ap = nc.dram_tensor("xgT", [160, EP], bf16, kind="ExternalInput").ap()
    esT_ap = nc.dram_tensor("esT", [12, EP], bf16, kind="ExternalInput").ap()
    eaT_ap = nc.dram_tensor("eaT", [128, (EP // 128) * 4], bf16,
                            kind="ExternalInput").ap()
    ohT_ap = nc.dram_tensor("ohT", [128, EP], bf16, kind="ExternalInput").ap()
    xscT_ap = nc.dram_tensor("xscT", [160, NPC], bf16, kind="ExternalInput").ap()
    Wnode_ap = nc.dram_tensor("Wnode", [160, 160], bf16, kind="ExternalInput").ap()
    Wsc_ap = nc.dram_tensor("Wsc", [160, 192], bf16, kind="ExternalInput").ap()
    Wr1_ap = nc.dram_tensor("Wr1p", [12, 100], bf16, kind="ExternalInput").ap()
    Wr2_ap = nc.dram_tensor("Wr2p", [100, 640], bf16, kind="ExternalInput").ap()
    W2p_ap = nc.dram_tensor("W2p", [640, 192], bf16, kind="ExternalInput").ap()
    out_ap = nc.dram_tensor("out", [NPC, 160], f32, kind="ExternalOutput").ap()

    with tile.TileContext(nc) as tc:
        from contextlib import ExitStack
        with ExitStack() as ctx:
            wpool = ctx.enter_context(tc.tile_pool(name="weights", bufs=1))

            wn1 = wpool.tile([128, 128], bf16)
            wn2 = wpool.tile([32, 32], bf16)
            ws1 = wpool.tile([128, 192], bf16)
            ws2 = wpool.tile([32, 192], bf16)
            wr1 = wpool.tile([12, 100], bf16)
            wr2 = wpool.tile([100, 640], bf16)
            w2p = [wpool.tile([128, 192], bf16, tag=f"w2p{j}", name=f"w2p{j}")
                   for j in range(5)]
            nc.sync.dma_start(wn1[:], Wnode_ap[0:128, 0:128])
            nc.sync.dma_start(wn2[:], Wnode_ap[128:160, 128:160])
            nc.sync.dma_start(wr1[:], Wr1_ap[:])

            ioti = wpool.tile([128, 128], i32)
            iotf = wpool.tile([128, 128], f32)
            iotci = wpool.tile([128, 1], i32)
            iotcf = wpool.tile([128, 1], f32)
            ident = wpool.tile([128, 128], bf16)
            nc.gpsimd.iota(ioti[:], pattern=[[1, 128]], base=0, channel_multiplier=0)
            nc.vector.tensor_copy(iotf[:], ioti[:])
            nc.gpsimd.iota(iotci[:], pattern=[[0, 1]], base=0, channel_multiplier=1)
            nc.vector.tensor_copy(iotcf[:], iotci[:])
            nc.vector.tensor_scalar(ident[:], iotf[:], iotcf[:], None,
                                    op0=mybir.AluOpType.is_equal)
            scN = wpool.tile([128, NWIN * 192], bf16)

            # Phase B: edges (self-connection interleaved per window)
            xsa = ctx.enter_context(tc.tile_pool(name="xsa", bufs=2))
            xsb = ctx.enter_context(tc.tile_pool(name="xsb", bufs=2))
            xg1P = ctx.enter_context(tc.tile_pool(name="xg1", bufs=2))
            xg2P = ctx.enter_context(tc.tile_pool(name="xg2", bufs=2))
            esP = ctx.enter_context(tc.tile_pool(name="esw", bufs=2))
            eaP = ctx.enter_context(tc.tile_pool(name="eaw", bufs=2))
            dsP = ctx.enter_context(tc.tile_pool(name="dsw", bufs=2))
            hsP = ctx.enter_context(tc.tile_pool(name="hs", bufs=16))
            gP = ctx.enter_context(tc.tile_pool(name="gp", bufs=1, space="PSUM"))
            hpP = ctx.enter_context(tc.tile_pool(name="hp", bufs=1, space="PSUM"))
            wpP = ctx.enter_context(tc.tile_pool(name="wp", bufs=1, space="PSUM"))
            wsP = ctx.enter_context(tc.tile_pool(name="wsb", bufs=2))
            mP = ctx.enter_context(tc.tile_pool(name="mid", bufs=2))
            gaP = ctx.enter_context(tc.tile_pool(name="ga", bufs=2))
            accP = ctx.enter_context(tc.tile_pool(name="acc", bufs=1, space="PSUM"))
            tlP = ctx.enter_context(tc.tile_pool(name="tail", bufs=2))
            tpsP = ctx.enter_context(tc.tile_pool(name="tps", bufs=1, space="PSUM"))
            ypP = ctx.enter_context(tc.tile_pool(name="yp", bufs=1, space="PSUM"))
            oP = ctx.enter_context(tc.tile_pool(name="outs", bufs=2))

            hs_cur = []
            for w in range(NWIN):
                xg1w = xg1P.tile([128, TW], bf16)
                xg2w = xg2P.tile([32, TW], bf16)
                nc.sync.dma_start(xg1w[:], xgT_ap[0:128, w * TW:(w + 1) * TW])
                nc.sync.dma_start(xg2w[:], xgT_ap[128:160, w * TW:(w + 1) * TW])
                eaw = eaP.tile([128, 4 * WT], bf16)
                nc.sync.dma_start(eaw[:], eaT_ap[:, w * 4 * WT:(w + 1) * 4 * WT])
                ohw = dsP.tile([128, TW], bf16)
                nc.sync.dma_start(ohw[:], ohT_ap[:, w * TW:(w + 1) * TW])

                # self-connection block for this window
                xs1 = xsa.tile([128, 128], bf16)
                xs2 = xsb.tile([32, 128], bf16)
                nc.sync.dma_start(xs1[:], xscT_ap[0:128, w * 128:(w + 1) * 128])
                nc.sync.dma_start(xs2[:], xscT_ap[128:160, w * 128:(w + 1) * 128])
                if w == 0:
                    nc.sync.dma_start(wr2[:], Wr2_ap[:])
                    nc.sync.dma_start(ws1[:], Wsc_ap[0:128, :])
                    nc.sync.dma_start(ws2[:], Wsc_ap[128:160, :])
                    for j in range(5):
                        nc.sync.dma_start(w2p[j][:], W2p_ap[j * 128:(j + 1) * 128, :])
                scp_t = ypP.tile([128, 192], f32, tag="y", name="scp_t")
                nc.tensor.matmul(scp_t[:], xs1[:], ws1[:], start=True, stop=False)
                nc.tensor.matmul(scp_t[:], xs2[:], ws2[:], start=False, stop=True)
                nc.scalar.activation(scN[:, w * 192:(w + 1) * 192], scp_t[:], AF.Copy)

                # radial-1 for a pair of windows at a time (fewer Silu<->Copy
                # activation-table swaps)
                if w % 2 == 0:
                    nwp = min(2, NWIN - w)
                    esw = esP.tile([12, nwp * TW], bf16, tag="esw")
                    nc.sync.dma_start(esw[:], esT_ap[:, w * TW:(w + nwp) * TW])
                    hs_cur = []
                    o = 0
                    while o < nwp * TW:
                        cw = min(512, nwp * TW - o)
                        hp = hpP.tile([100, 512], f32, tag="hp")
                        nc.tensor.matmul(hp[:, 0:cw], wr1[:], esw[:, o:o + cw],
                                         start=True, stop=True)
                        hc = hsP.tile([100, 512], bf16, tag="hs")
                        nc.scalar.activation(hc[:, 0:cw], hp[:, 0:cw], AF.Silu)
                        hs_cur.append(hc)
                        o += cw

                acc0 = accP.tile([128, 320], f32, tag="acc0")
                acc1 = accP.tile([128, 320], f32, tag="acc1")
                for t in range(WT):
                    gp = gP.tile([128, 160], f32)
                    nc.tensor.matmul(gp[:, 0:128], xg1w[:, t * 128:(t + 1) * 128],
                                     wn1[:], start=True, stop=True)
                    nc.tensor.matmul(gp[:, 128:160], xg2w[:, t * 128:(t + 1) * 128],
                                     wn2[:], start=True, stop=True)

                    wp0 = wpP.tile([128, 320], f32, tag="wp0")
                    wp1 = wpP.tile([128, 320], f32, tag="wp1")
                    gt = (w % 2) * WT + t
                    hchunk = hs_cur[gt // 4]
                    hsl = hchunk[:, (gt % 4) * 128:(gt % 4 + 1) * 128]
                    nc.tensor.matmul(wp0[:], hsl, wr2[:, 0:320],
                                     start=True, stop=True)
                    nc.tensor.matmul(wp1[:], hsl, wr2[:, 320:640],
                                     start=True, stop=True)
                    wsb = wsP.tile([128, 640], bf16)
                    nc.scalar.activation(wsb[:, 0:320], wp0[:], AF.Copy)
                    nc.scalar.activation(wsb[:, 320:640], wp1[:], AF.Copy)
                    ga = gaP.tile([128, 640], bf16)
                    g_b = gp[:].unsqueeze(1).broadcast_to([128, 4, 160])
                    a_b = eaw[:, 4 * t:4 * t + 4].unsqueeze(2).broadcast_to(
                        [128, 4, 160])
                    nc.vector.tensor_tensor(
                        ga[:].rearrange("p (b f) -> p b f", b=4), g_b, a_b, MUL)
                    mid = mP.tile([128, 640], bf16)
                    nc.vector.tensor_tensor(mid[:], wsb[:], ga[:], MUL)
                    ohsl = ohw[:, t * 128:(t + 1) * 128]
                    nc.tensor.matmul(acc0[:], ohsl, mid[:, 0:320],
                                     start=(t == 0), stop=(t == WT - 1))
                    nc.tensor.matmul(acc1[:], ohsl, mid[:, 320:640],
                                     start=(t == 0), stop=(t == WT - 1))

                # window tail: lin2 + sc + gate
                asb = tlP.tile([128, 640], bf16, tag="asb")
                nc.scalar.activation(asb[:, 0:320], acc0[:], AF.Copy)
                nc.scalar.activation(asb[:, 320:640], acc1[:], AF.Copy)
                yp = ypP.tile([128, 192], f32, tag="y", name="yp")
                for j in range(5):
                    tp = tpsP.tile([128, 128], bf16, tag="tp")
                    nc.tensor.transpose(tp[:], asb[:, j * 128:(j + 1) * 128], ident[:])
                    ts = tlP.tile([128, 128], bf16, tag="ts")
                    nc.scalar.activation(ts[:], tp[:], AF.Copy)
                    nc.tensor.matmul(yp[:], ts[:], w2p[j][:],
                                     start=(j == 0), stop=(j == 4))
                y2 = tlP.tile([128, 192], bf16, tag="y2")
                nc.vector.tensor_add(y2[:], yp[:], scN[:, w * 192:(w + 1) * 192])
                outt = oP.tile([128, 160], f32, tag="outt")
                gtl = oP.tile([128, 32], bf16, tag="gtl")
                sgo = oP.tile([128, 64], bf16, tag="sgo")
                nc.scalar.activation(sgo[:], y2[:, 0:64], AF.Sigmoid)
                nc.vector.tensor_mul(outt[:, 0:64], y2[:, 0:64], sgo[:])
                nc.scalar.activation(gtl[:], y2[:, 64:96], AF.Sigmoid)
                gtl3 = gtl[:].unsqueeze(1).broadcast_to([128, 3, 32])
                nc.vector.tensor_tensor(
                    outt[:, 64:160].rearrange("p (c u) -> p c u", c=3),
                    y2[:, 96:192].rearrange("p (c u) -> p c u", c=3), gtl3, MUL)
                nc.sync.dma_start(out_ap[w * 128:(w + 1) * 128, :], outt[:])

    nc.compile()
    return nc


def kernel(x, z, edge_src, edge_dst, edge_attr, edge_scalars,
           W_sc_s, W_sc_v, W1_s, W1_v, W_r1, W_r2, W2_s, W2_v):
    from concourse import bass_utils
    x = np.asarray(x, np.float32)
    edge_src = np.asarray(edge_src, np.int64)
    edge_dst = np.asarray(edge_dst, np.int64)
    edge_attr = np.asarray(edge_attr, np.float32)
    edge_scalars = np.asarray(edge_scalars, np.float32)

    # global balanced node->(core,window) binning; uniform WT (SPMD program)
    core, slot, wsum = _assign_slots(edge_dst)
    WT = int(np.ceil(wsum.max() / 128.0))

    key = WT
    if key not in _CACHE:
        _CACHE[key] = _build_program(WT)
    nc = _CACHE[key]

    Wnode, Wsc, Wr1p, Wr2p, W2p = _prep_weights(
        np.asarray(W_sc_s, np.float32), np.asarray(W_sc_v, np.float32),
        np.asarray(W1_s, np.float32), np.asarray(W1_v, np.float32),
        np.asarray(W_r1, np.float32), np.asarray(W_r2, np.float32),
        np.asarray(W2_s, np.float32), np.asarray(W2_v, np.float32))

    in_maps = []
    for c in range(NCORES):
        m = _prep_core(c, x, edge_src, edge_dst, edge_attr, edge_scalars, WT,
                       core, slot)
        m.update(Wnode=Wnode, Wsc=Wsc, Wr1p=Wr1p, Wr2p=Wr2p, W2p=W2p)
        in_maps.append(m)

    res = bass_utils.run_bass_kernel_spmd(nc, in_maps, core_ids=list(range(NCORES)))
    full = np.empty((N, 160), np.float32)
    for c in range(NCORES):
        own = np.nonzero(core == c)[0]
        full[own] = res.results[c]["out"][slot[own]]
    out = np.empty((N, 160), np.float32)
    out[:, 0:64] = full[:, 0:64]
    # device gated layout is c-major [32c+u]; reference wants u-major [3u+c]
    out[:, 64:160] = full[:, 64:160].reshape(N, 3, 32).transpose(0, 2, 1).reshape(N, 96)
    return out
